# revision 1
# baseline (speedup 1.0000x reference)
"""KMeans assignment kernel for Trainium2 (8 NeuronCores, SPMD data-parallel).

Problem: x [8, 4096, 1024] f32, C [1024, 4096] f32, Cnorm [1, 4096] f32.
Output: argmin_k(|x|^2 - 2 x.C + Cnorm) as int32 [8, 4096].

Strategy:
  - |x|^2 is row-constant, so argmin(dist) == argmax(x.C - 0.5*Cnorm).
  - Shard rows (N = B*T = 32768) across 8 cores, 4096 rows each; replicate C.
  - Host pre-transposes x into [D, rows] tiles (the PE contracts along the
    partition dim, so the stationary operand is x^T).

Two kernel modes (MODE below):
  - "f32r" (default): single-pass fp22-truncated f32 matmul at full PE rate
    (1 cyc/row).  fp22 truncation noise is sigma ~ 4.7e-3 (measured on HW)
    per score, so the kernel also emits the top1-top2 margin per row; rows
    with margin < TAU (~12 sigma) are recomputed exactly on the host with
    the reference's own jax-on-CPU numerics (~0.8% of rows).  Epilogue is
    spread across the otherwise-idle engines: ACT copies PSUM->SBUF, GPSIMD
    subtracts 0.5*Cnorm in place, DVE does MAX8/FIND_INDEX8 only.
  - "bf16x3": 3 bf16 passes (x_hi.C_hi + x_hi.C_lo + x_lo.C_hi with exact
    bf16 splits).  PE bf16 products are exact (e10m23 accumulate), giving
    ~2^-18 relative error -- bit-stable argmins, no host fixup, ~3x slower.
"""

import os
import sys

import numpy as np
import ml_dtypes

for _p in ("/opt/trn_rl_repo",):
    if os.path.isdir(_p) and _p not in sys.path:
        sys.path.insert(0, _p)

import concourse.bass as bass
import concourse.mybir as mybir
import concourse.tile as tile
from concourse import bacc
from concourse.bass_utils import run_bass_kernel_spmd

BF16 = ml_dtypes.bfloat16

B, T, D, K = 8, 4096, 1024, 4096
N_CORES = 8
ROWS = (B * T) // N_CORES  # 4096 rows per core
P = 128  # SBUF partitions / PE tile
MT = ROWS // P  # 32 row-tiles per core
DC = D // P  # 8 contraction chunks
NB = 512  # matmul free dim = one PSUM bank of f32
NC_ = K // NB  # 8 centroid chunks

MODE = os.environ.get("KMEANS_KERNEL_MODE", "f32r")
TAU = 0.08  # score-margin flag threshold (~12 sigma of fp22 noise)

_compiled = {}


def _build_f32r():
    nc = bacc.Bacc("TRN2", target_bir_lowering=False, debug=False, num_devices=N_CORES)

    x_d = nc.dram_tensor("x", [MT, DC, P, P], mybir.dt.float32r, kind="ExternalInput")
    c_d = nc.dram_tensor("c", [DC, P, K], mybir.dt.float32r, kind="ExternalInput")
    cn_d = nc.dram_tensor("cn", [P, K], mybir.dt.float32, kind="ExternalInput")
    out_d = nc.dram_tensor("out", [ROWS], mybir.dt.uint32, kind="ExternalOutput")
    marg_d = nc.dram_tensor("marg", [ROWS], mybir.dt.float32, kind="ExternalOutput")

    with tile.TileContext(nc) as tc:
        with (
            tc.tile_pool(name="const", bufs=1) as cpool,
            tc.tile_pool(name="xp", bufs=3) as xpool,
            tc.tile_pool(name="sc", bufs=2) as spool,
            tc.tile_pool(name="ixp", bufs=4) as ipool,
            tc.tile_pool(name="ps", bufs=NC_, space=bass.MemorySpace.PSUM) as ppool,
        ):
            c_sb = cpool.tile([P, DC, K], mybir.dt.float32r, tag="c")
            cn_sb = cpool.tile([P, K], mybir.dt.float32, tag="cn")
            for c in range(DC):
                nc.sync.dma_start(out=c_sb[:, c, :], in_=c_d[c])
            nc.sync.dma_start(out=cn_sb[:], in_=cn_d[:])

            for m in range(MT):
                x_sb = xpool.tile([P, DC, P], mybir.dt.float32r, tag="x")
                nc.sync.dma_start(out=x_sb[:], in_=x_d[m].rearrange("c p j -> p c j"))

                psum_tiles = [
                    ppool.tile([P, NB], mybir.dt.float32, tag="ps", name=f"ps{m}_{n}")
                    for n in range(NC_)
                ]
                for c in range(DC):
                    for n in range(NC_):
                        nc.tensor.matmul(
                            psum_tiles[n][:],
                            x_sb[:, c, :],
                            c_sb[:, c, n * NB : (n + 1) * NB],
                            start=(c == 0),
                            stop=(c == DC - 1),
                        )

                score_sb = spool.tile([P, K], mybir.dt.float32, tag="score")
                for n in range(NC_):
                    sl = slice(n * NB, (n + 1) * NB)
                    # ACT drains PSUM; GPSIMD applies the -0.5*Cnorm bias.
                    nc.scalar.copy(score_sb[:, sl], psum_tiles[n][:])
                    nc.gpsimd.tensor_sub(score_sb[:, sl], score_sb[:, sl], cn_sb[:, sl])

                mx = ipool.tile([P, 8], mybir.dt.float32, tag="mx")
                ix = ipool.tile([P, 8], mybir.dt.uint32, tag="ix")
                mg = ipool.tile([P, 1], mybir.dt.float32, tag="mg")
                nc.vector.max(out=mx[:], in_=score_sb[:])
                nc.vector.max_index(ix[:], mx[:], score_sb[:])
                nc.vector.tensor_sub(mg[:], mx[:, 0:1], mx[:, 1:2])

                nc.sync.dma_start(out=out_d[m * P : (m + 1) * P], in_=ix[:, 0:1])
                nc.sync.dma_start(out=marg_d[m * P : (m + 1) * P], in_=mg[:])

    nc.compile()
    return nc


def _build_bf16x3():
    nc = bacc.Bacc("TRN2", target_bir_lowering=False, debug=False, num_devices=N_CORES)

    xhi_d = nc.dram_tensor("xhi", [MT, DC, P, P], mybir.dt.bfloat16, kind="ExternalInput")
    xlo_d = nc.dram_tensor("xlo", [MT, DC, P, P], mybir.dt.bfloat16, kind="ExternalInput")
    chi_d = nc.dram_tensor("chi", [DC, P, K], mybir.dt.bfloat16, kind="ExternalInput")
    clo_d = nc.dram_tensor("clo", [DC, P, K], mybir.dt.bfloat16, kind="ExternalInput")
    cn_d = nc.dram_tensor("cn", [P, K], mybir.dt.float32, kind="ExternalInput")
    out_d = nc.dram_tensor("out", [ROWS], mybir.dt.uint32, kind="ExternalOutput")

    with tile.TileContext(nc) as tc:
        with (
            tc.tile_pool(name="const", bufs=1) as cpool,
            tc.tile_pool(name="xp", bufs=3) as xpool,
            tc.tile_pool(name="sc", bufs=2) as spool,
            tc.tile_pool(name="ixp", bufs=4) as ipool,
            tc.tile_pool(name="ps", bufs=NC_, space=bass.MemorySpace.PSUM) as ppool,
        ):
            chi_sb = cpool.tile([P, DC, K], mybir.dt.bfloat16, tag="chi")
            clo_sb = cpool.tile([P, DC, K], mybir.dt.bfloat16, tag="clo")
            cn_sb = cpool.tile([P, K], mybir.dt.float32, tag="cn")
            for c in range(DC):
                nc.sync.dma_start(out=chi_sb[:, c, :], in_=chi_d[c])
                nc.sync.dma_start(out=clo_sb[:, c, :], in_=clo_d[c])
            nc.sync.dma_start(out=cn_sb[:], in_=cn_d[:])

            for m in range(MT):
                xhi_sb = xpool.tile([P, DC, P], mybir.dt.bfloat16, tag="xhi")
                xlo_sb = xpool.tile([P, DC, P], mybir.dt.bfloat16, tag="xlo")
                nc.sync.dma_start(out=xhi_sb[:], in_=xhi_d[m].rearrange("c p j -> p c j"))
                nc.sync.dma_start(out=xlo_sb[:], in_=xlo_d[m].rearrange("c p j -> p c j"))

                psum_tiles = [
                    ppool.tile([P, NB], mybir.dt.float32, tag="ps", name=f"ps{m}_{n}")
                    for n in range(NC_)
                ]

                wlist = []
                for xsb, csb in ((xhi_sb, chi_sb), (xhi_sb, clo_sb), (xlo_sb, chi_sb)):
                    for c in range(DC):
                        wlist.append((xsb[:, c, :], csb, c))
                nw = len(wlist)
                for wi, (lhs, csb, c) in enumerate(wlist):
                    for n in range(NC_):
                        nc.tensor.matmul(
                            psum_tiles[n][:],
                            lhs,
                            csb[:, c, n * NB : (n + 1) * NB],
                            start=(wi == 0),
                            stop=(wi == nw - 1),
                        )

                score_sb = spool.tile([P, K], mybir.dt.float32, tag="score")
                for n in range(NC_):
                    nc.vector.tensor_sub(
                        score_sb[:, n * NB : (n + 1) * NB],
                        psum_tiles[n][:],
                        cn_sb[:, n * NB : (n + 1) * NB],
                    )

                mx = ipool.tile([P, 8], mybir.dt.float32, tag="mx")
                ix = ipool.tile([P, 8], mybir.dt.uint32, tag="ix")
                nc.vector.max(out=mx[:], in_=score_sb[:])
                nc.vector.max_index(ix[:], mx[:], score_sb[:])

                nc.sync.dma_start(out=out_d[m * P : (m + 1) * P], in_=ix[:, 0:1])

    nc.compile()
    return nc


def _xt_tiles(xs, dtype):
    # [r, d] -> [m, c, p, j] with r = m*128 + j, d = c*128 + p
    return np.ascontiguousarray(
        xs.astype(dtype).reshape(MT, P, DC, P).transpose(0, 2, 3, 1)
    )


def _prep_f32r(x2, Cf, cn):
    c3 = np.ascontiguousarray(Cf.reshape(DC, P, K))
    in_maps = []
    for s in range(N_CORES):
        xs = x2[s * ROWS : (s + 1) * ROWS]
        in_maps.append({"x": _xt_tiles(xs, np.float32), "c": c3, "cn": cn})
    return in_maps


def _prep_bf16x3(x2, Cf, cn):
    Chi = Cf.astype(BF16)
    Clo = (Cf - Chi.astype(np.float32)).astype(BF16)
    chi = np.ascontiguousarray(Chi.reshape(DC, P, K))
    clo = np.ascontiguousarray(Clo.reshape(DC, P, K))
    in_maps = []
    for s in range(N_CORES):
        xs = x2[s * ROWS : (s + 1) * ROWS]
        xhi = xs.astype(BF16)
        xlo = (xs - xhi.astype(np.float32)).astype(BF16)
        in_maps.append(
            {
                "xhi": _xt_tiles(xhi, BF16),
                "xlo": _xt_tiles(xlo, BF16),
                "chi": chi,
                "clo": clo,
                "cn": cn,
            }
        )
    return in_maps


def _host_fixup(assigned, margins, x2, Cf, Cnorm):
    """Recompute rows whose fp22 score margin is within noise of a tie,
    replicating the reference's jax-on-CPU f32 numerics exactly."""
    bad = np.flatnonzero(margins < TAU)
    if bad.size == 0:
        return assigned
    import jax
    import jax.numpy as jnp

    cpu = jax.devices("cpu")[0]
    with jax.default_device(cpu):
        xb = jnp.asarray(x2[bad])
        Cj = jnp.asarray(Cf)
        cnj = jnp.asarray(Cnorm.reshape(1, K))
        dist = jnp.sum(xb * xb, axis=1, keepdims=True) - 2.0 * (xb @ Cj) + cnj
        fixed = np.asarray(jnp.argmin(dist, axis=1), dtype=assigned.dtype)
    assigned[bad] = fixed
    return assigned


def run(inputs, trace=False, mode=None):
    """Returns (assigned [B, T] int32, BassKernelResults)."""
    mode = mode or MODE
    if mode not in _compiled:
        _compiled[mode] = _build_f32r() if mode == "f32r" else _build_bf16x3()
    nc = _compiled[mode]

    x2 = np.ascontiguousarray(
        np.asarray(inputs["x"], dtype=np.float32).reshape(B * T, D)
    )
    Cf = np.ascontiguousarray(np.asarray(inputs["C"], dtype=np.float32))
    Cnorm = np.asarray(inputs["Cnorm"], dtype=np.float32)
    cn = np.ascontiguousarray(
        np.broadcast_to(0.5 * Cnorm.reshape(1, K), (P, K)).astype(np.float32)
    )

    if mode == "f32r":
        in_maps = _prep_f32r(x2, Cf, cn)
    else:
        in_maps = _prep_bf16x3(x2, Cf, cn)

    res = run_bass_kernel_spmd(nc, in_maps, list(range(N_CORES)), trace=trace)

    assigned = np.concatenate(
        [np.asarray(res.results[s]["out"]).reshape(ROWS) for s in range(N_CORES)]
    ).astype(np.int32)
    if mode == "f32r":
        margins = np.concatenate(
            [np.asarray(res.results[s]["marg"]).reshape(ROWS) for s in range(N_CORES)]
        )
        assigned = _host_fixup(assigned, margins, x2, Cf, Cnorm)
    return assigned.reshape(B, T), res


def kernel(x, C, Cnorm):
    assigned, _ = run({"x": x, "C": C, "Cnorm": Cnorm})
    return assigned



# revision 7
# speedup vs baseline: 1.6819x; 1.6819x over previous
"""KMeans assignment kernel for Trainium2 (8 NeuronCores, SPMD data-parallel).

Problem: x [8, 4096, 1024] f32, C [1024, 4096] f32, Cnorm [1, 4096] f32.
Output: argmin_k(|x|^2 - 2 x.C + Cnorm) as int32 [8, 4096].

Strategy:
  - |x|^2 is row-constant, so argmin(dist) == argmax(x.C - 0.5*Cnorm).
  - Shard rows (N = B*T = 32768) across 8 cores, 4096 rows each; replicate C.

Modes (KMEANS_KERNEL_MODE):
  - "fp8dr" (default): single fp8-e4m3 DoubleRow pass at 2x PE rate.
    Per 4-bank PSUM half-tile (2048 centroids): 4 fp16 "bias matmuls"
    (delta-matrix x bias-row, start=True) seed PSUM with -0.5*Cnorm, then
    16 fp8 DR matmuls accumulate q(x).q(C).  DVE MAX8/FIND_INDEX8 read the
    biased scores straight from PSUM -> per-half top-8 values+indices.
    Host merges the 2x8 candidates (a superset of the global top-8 since
    any global-top-8 score is top-8 within its own half), rescores them in
    exact f32, and fully rescores the ~0.2% of rows whose fp8 top1-top8
    margin is within noise (TAU8) or whose exact top1-top2 margin is a
    rounding-level tie (EPS_TIE), using the reference's jax-on-CPU numerics.
  - "f32r": single-pass fp22-truncated f32 matmul (1 cyc/row) + host fixup
    of rows with top1-top2 margin < TAU (~12 sigma of fp22 noise).
"""

import os
import sys

import numpy as np
import ml_dtypes

for _p in ("/opt/trn_rl_repo",):
    if os.path.isdir(_p) and _p not in sys.path:
        sys.path.insert(0, _p)

import concourse.bass as bass
import concourse.mybir as mybir
import concourse.tile as tile
from concourse import bacc
from concourse.bass_utils import run_bass_kernel_spmd

FP8 = ml_dtypes.float8_e4m3fn

B, T, D, K = 8, 4096, 1024, 4096
N_CORES = 8
ROWS = (B * T) // N_CORES  # 4096 rows per core
P = 128  # SBUF partitions / PE tile
MT = ROWS // P  # 32 row-tiles per core
NB = 512  # one PSUM bank of f32
QC = D // 256  # 4 DoubleRow contraction chunks (256 dims each)
NH = 2  # PSUM half-tiles per row-tile (4 banks each)
KH = K // NH  # 2048 centroids per half
NBH = KH // NB  # 4 PSUM banks per half

DC = D // P  # 8 contraction chunks (f32r mode)
NC_ = K // NB  # 8 centroid chunks (f32r mode)

MODE = os.environ.get("KMEANS_KERNEL_MODE", "fp8dr")
TAU = 0.08  # f32r: score-margin flag threshold (~12 sigma of fp22 noise)
TAU8 = 6.0  # fp8dr: top1-top8 fp8-margin flag threshold (~3.5 sigma)
EPS_TIE = 1e-2  # fp8dr: exact-rescore top1-top2 tie threshold

_compiled = {}


def _build_fp8dr():
    nc = bacc.Bacc("TRN2", target_bir_lowering=False, debug=False, num_devices=N_CORES)

    x_d = nc.dram_tensor("x", [MT, P, QC, 2, P], mybir.dt.float8e4, kind="ExternalInput")
    c_d = nc.dram_tensor("c", [P, QC, 2, K], mybir.dt.float8e4, kind="ExternalInput")
    bias_d = nc.dram_tensor("bias", [P, K], mybir.dt.float16, kind="ExternalInput")
    delta_d = nc.dram_tensor("delta", [P, P], mybir.dt.float16, kind="ExternalInput")
    mx_d = nc.dram_tensor("mx", [MT, NH, P, 8], mybir.dt.float32, kind="ExternalOutput")
    ix_d = nc.dram_tensor("ix", [MT, NH, P, 8], mybir.dt.uint32, kind="ExternalOutput")

    with tile.TileContext(nc) as tc:
        with (
            tc.tile_pool(name="const", bufs=1) as cpool,
            tc.tile_pool(name="xp", bufs=3) as xpool,
            tc.tile_pool(name="ixp", bufs=6) as ipool,
            tc.tile_pool(name="ps", bufs=2, space=bass.MemorySpace.PSUM) as ppool,
        ):
            # HAM warmup fodder: zeroed fp8 tile, harmless matmuls during DMA wait.
            warm_sb = cpool.tile([P, NB], mybir.dt.float8e4, tag="warm")
            nc.vector.memset(warm_sb[:], 0)

            c_sb = cpool.tile([P, QC, 2, K], mybir.dt.float8e4, tag="c")
            bias_sb = cpool.tile([P, K], mybir.dt.float16, tag="bias")
            delta_sb = cpool.tile([P, P], mybir.dt.float16, tag="delta")
            nc.sync.dma_start(out=delta_sb[:], in_=delta_d[:])
            nc.sync.dma_start(out=bias_sb[:], in_=bias_d[:])
            for c in range(QC):
                nc.sync.dma_start(out=c_sb[:, c], in_=c_d[:, c])

            warm_ps = ppool.tile([P, KH], mybir.dt.float32, tag="ps", name="warm")
            for w in range(24):
                nc.tensor.matmul(
                    warm_ps[:, :NB],
                    warm_sb[:, :P],
                    warm_sb[:],
                    start=True,
                    stop=True,
                )

            for m in range(MT):
                x_sb = xpool.tile([P, QC, 2, P], mybir.dt.float8e4, tag="x")
                nc.sync.dma_start(out=x_sb[:], in_=x_d[m])

                for h in range(NH):
                    ps = ppool.tile(
                        [P, KH], mybir.dt.float32, tag="ps", name=f"ps{m}_{h}"
                    )
                    for nb in range(NBH):
                        sl = slice(h * KH + nb * NB, h * KH + (nb + 1) * NB)
                        nc.tensor.matmul(
                            ps[:, nb * NB : (nb + 1) * NB],
                            delta_sb[:],
                            bias_sb[:, sl],
                            start=True,
                            stop=False,
                            skip_group_check=True,
                        )
                    for c in range(QC):
                        for nb in range(NBH):
                            sl = slice(h * KH + nb * NB, h * KH + (nb + 1) * NB)
                            nc.tensor.matmul(
                                ps[:, nb * NB : (nb + 1) * NB],
                                x_sb[:, c],
                                c_sb[:, c, :, sl],
                                start=False,
                                stop=(c == QC - 1),
                                perf_mode=mybir.MatmulPerfMode.DoubleRow,
                                skip_group_check=True,
                            )

                    mx = ipool.tile([P, 8], mybir.dt.float32, tag="mx")
                    ix = ipool.tile([P, 8], mybir.dt.uint32, tag="ix")
                    nc.vector.max(out=mx[:], in_=ps[:])
                    nc.vector.max_index(ix[:], mx[:], ps[:])
                    nc.sync.dma_start(out=mx_d[m, h], in_=mx[:])
                    nc.sync.dma_start(out=ix_d[m, h], in_=ix[:])

    nc.compile()
    return nc


def _build_f32r():
    nc = bacc.Bacc("TRN2", target_bir_lowering=False, debug=False, num_devices=N_CORES)

    x_d = nc.dram_tensor("x", [MT, DC, P, P], mybir.dt.float32r, kind="ExternalInput")
    c_d = nc.dram_tensor("c", [DC, P, K], mybir.dt.float32r, kind="ExternalInput")
    cn_d = nc.dram_tensor("cn", [P, K], mybir.dt.float32, kind="ExternalInput")
    out_d = nc.dram_tensor("out", [ROWS], mybir.dt.uint32, kind="ExternalOutput")
    marg_d = nc.dram_tensor("marg", [ROWS], mybir.dt.float32, kind="ExternalOutput")

    with tile.TileContext(nc) as tc:
        with (
            tc.tile_pool(name="const", bufs=1) as cpool,
            tc.tile_pool(name="xp", bufs=3) as xpool,
            tc.tile_pool(name="sc", bufs=2) as spool,
            tc.tile_pool(name="ixp", bufs=4) as ipool,
            tc.tile_pool(name="ps", bufs=NC_, space=bass.MemorySpace.PSUM) as ppool,
        ):
            c_sb = cpool.tile([P, DC, K], mybir.dt.float32r, tag="c")
            cn_sb = cpool.tile([P, K], mybir.dt.float32, tag="cn")
            for c in range(DC):
                nc.sync.dma_start(out=c_sb[:, c, :], in_=c_d[c])
            nc.sync.dma_start(out=cn_sb[:], in_=cn_d[:])

            for m in range(MT):
                x_sb = xpool.tile([P, DC, P], mybir.dt.float32r, tag="x")
                nc.sync.dma_start(out=x_sb[:], in_=x_d[m].rearrange("c p j -> p c j"))

                psum_tiles = [
                    ppool.tile([P, NB], mybir.dt.float32, tag="ps", name=f"ps{m}_{n}")
                    for n in range(NC_)
                ]
                for c in range(DC):
                    for n in range(NC_):
                        nc.tensor.matmul(
                            psum_tiles[n][:],
                            x_sb[:, c, :],
                            c_sb[:, c, n * NB : (n + 1) * NB],
                            start=(c == 0),
                            stop=(c == DC - 1),
                        )

                score_sb = spool.tile([P, K], mybir.dt.float32, tag="score")
                for n in range(NC_):
                    sl = slice(n * NB, (n + 1) * NB)
                    nc.scalar.copy(score_sb[:, sl], psum_tiles[n][:])
                    nc.gpsimd.tensor_sub(score_sb[:, sl], score_sb[:, sl], cn_sb[:, sl])

                mx = ipool.tile([P, 8], mybir.dt.float32, tag="mx")
                ix = ipool.tile([P, 8], mybir.dt.uint32, tag="ix")
                mg = ipool.tile([P, 1], mybir.dt.float32, tag="mg")
                nc.vector.max(out=mx[:], in_=score_sb[:])
                nc.vector.max_index(ix[:], mx[:], score_sb[:])
                nc.vector.tensor_sub(mg[:], mx[:, 0:1], mx[:, 1:2])

                nc.sync.dma_start(out=out_d[m * P : (m + 1) * P], in_=ix[:, 0:1])
                nc.sync.dma_start(out=marg_d[m * P : (m + 1) * P], in_=mg[:])

    nc.compile()
    return nc


def _prep_fp8dr(x2, Cf, Cnorm):
    # C: [d, k] -> [p, c, i, k] with d = c*256 + i*128 + p
    cq = np.ascontiguousarray(
        Cf.astype(FP8).reshape(QC, 2, P, K).transpose(2, 0, 1, 3)
    )
    bias = np.zeros((P, K), np.float16)
    bias[0, :] = (-0.5 * Cnorm.reshape(K)).astype(np.float16)
    delta = np.zeros((P, P), np.float16)
    delta[0, :] = 1.0

    in_maps = []
    for s in range(N_CORES):
        xs = x2[s * ROWS : (s + 1) * ROWS].astype(FP8)
        # [rows, d] -> [m, p, c, i, j] with rows = m*128 + j, d = c*256 + i*128 + p
        xt = np.ascontiguousarray(
            xs.reshape(MT, P, QC, 2, P).transpose(0, 4, 2, 3, 1)
        )
        in_maps.append({"x": xt, "c": cq, "bias": bias, "delta": delta})
    return in_maps


def _xt_tiles(xs, dtype):
    # [r, d] -> [m, c, p, j] with r = m*128 + j, d = c*128 + p
    return np.ascontiguousarray(
        xs.astype(dtype).reshape(MT, P, DC, P).transpose(0, 2, 3, 1)
    )


def _prep_f32r(x2, Cf, cn):
    c3 = np.ascontiguousarray(Cf.reshape(DC, P, K))
    in_maps = []
    for s in range(N_CORES):
        xs = x2[s * ROWS : (s + 1) * ROWS]
        in_maps.append({"x": _xt_tiles(xs, np.float32), "c": c3, "cn": cn})
    return in_maps


def _full_rescore(rows, x2, Cf, Cnorm):
    """Exact per-row argmin over all K, replicating the reference's
    jax-on-CPU f32 numerics."""
    import jax
    import jax.numpy as jnp

    cpu = jax.devices("cpu")[0]
    with jax.default_device(cpu):
        xb = jnp.asarray(x2[rows])
        Cj = jnp.asarray(Cf)
        cnj = jnp.asarray(Cnorm.reshape(1, K))
        dist = jnp.sum(xb * xb, axis=1, keepdims=True) - 2.0 * (xb @ Cj) + cnj
        return np.asarray(jnp.argmin(dist, axis=1))


def _decode_fp8dr(res, x2, Cf, Cnorm):
    N = B * T
    vals = np.concatenate(
        [
            np.asarray(res.results[s]["mx"]).transpose(0, 2, 1, 3).reshape(ROWS, 16)
            for s in range(N_CORES)
        ]
    )  # [N, 16]: per row, (half0 top8, half1 top8) after transpose -> [m, j, h, 8]
    idxl = np.concatenate(
        [
            np.asarray(res.results[s]["ix"])
            .astype(np.int64)
            .transpose(0, 2, 1, 3)
            .reshape(ROWS, 16)
            for s in range(N_CORES)
        ]
    )
    # local index within half -> global centroid id
    half = np.tile(np.repeat(np.arange(NH), 8), (N, 1))
    cands = idxl + half * KH  # [N, 16]
    np.clip(cands, 0, K - 1, out=cands)

    # exact f32 rescore of the 16 candidates, in distance space
    CT = np.ascontiguousarray(Cf.T)  # [K, D]
    cn = Cnorm.reshape(K)
    pick = np.zeros(N, np.int64)
    tie = np.zeros(N, np.float32)
    CH = 4096
    for i in range(0, N, CH):
        cd = cands[i : i + CH]
        d = cn[cd] - 2.0 * np.einsum(
            "nkd,nd->nk", CT[cd], x2[i : i + CH], optimize=True
        )
        j = np.argmin(d, axis=1)
        pick[i : i + CH] = cd[np.arange(len(cd)), j]
        ds = np.sort(d, axis=1)
        tie[i : i + CH] = ds[:, 1] - ds[:, 0]

    # flag rows where fp8 noise or f32 rounding could flip the argmin
    vs = -np.sort(-vals, axis=1)
    gap8 = vs[:, 0] - vs[:, 7]
    flagged = np.flatnonzero((gap8 < TAU8) | (tie < EPS_TIE))
    if flagged.size:
        pick[flagged] = _full_rescore(flagged, x2, Cf, Cnorm)
    return pick


def _host_fixup_f32r(assigned, margins, x2, Cf, Cnorm):
    bad = np.flatnonzero(margins < TAU)
    if bad.size == 0:
        return assigned
    assigned[bad] = _full_rescore(bad, x2, Cf, Cnorm).astype(assigned.dtype)
    return assigned


def run(inputs, trace=False, mode=None):
    """Returns (assigned [B, T] int32, BassKernelResults)."""
    mode = mode or MODE
    if mode not in _compiled:
        _compiled[mode] = _build_fp8dr() if mode == "fp8dr" else _build_f32r()
    nc = _compiled[mode]

    x2 = np.ascontiguousarray(
        np.asarray(inputs["x"], dtype=np.float32).reshape(B * T, D)
    )
    Cf = np.ascontiguousarray(np.asarray(inputs["C"], dtype=np.float32))
    Cnorm = np.asarray(inputs["Cnorm"], dtype=np.float32)

    if mode == "fp8dr":
        in_maps = _prep_fp8dr(x2, Cf, Cnorm)
    else:
        cn = np.ascontiguousarray(
            np.broadcast_to(0.5 * Cnorm.reshape(1, K), (P, K)).astype(np.float32)
        )
        in_maps = _prep_f32r(x2, Cf, cn)

    res = run_bass_kernel_spmd(nc, in_maps, list(range(N_CORES)), trace=trace)

    if mode == "fp8dr":
        assigned = _decode_fp8dr(res, x2, Cf, Cnorm).astype(np.int32)
    else:
        assigned = np.concatenate(
            [np.asarray(res.results[s]["out"]).reshape(ROWS) for s in range(N_CORES)]
        ).astype(np.int32)
        margins = np.concatenate(
            [np.asarray(res.results[s]["marg"]).reshape(ROWS) for s in range(N_CORES)]
        )
        assigned = _host_fixup_f32r(assigned, margins, x2, Cf, Cnorm)
    return assigned.reshape(B, T), res


def kernel(x, C, Cnorm):
    assigned, _ = run({"x": x, "C": C, "Cnorm": Cnorm})
    return assigned


# revision 15
# speedup vs baseline: 1.7572x; 1.0448x over previous
"""KMeans assignment kernel for Trainium2 (8 NeuronCores, SPMD data-parallel).

Problem: x [8, 4096, 1024] f32, C [1024, 4096] f32, Cnorm [1, 4096] f32.
Output: argmin_k(|x|^2 - 2 x.C + Cnorm) as int32 [8, 4096].

Strategy:
  - |x|^2 is row-constant, so argmin(dist) == argmax(x.C - 0.5*Cnorm).
  - Shard rows (N = B*T = 32768) across 8 cores, 4096 rows each; replicate C.

Modes (KMEANS_KERNEL_MODE):
  - "fp8dr" (default): single fp8-e4m3 DoubleRow pass at 2x PE rate.
    Per 4-bank PSUM half-tile (2048 centroids): 4 fp16 "bias matmuls"
    (delta-matrix x bias-row, start=True) seed PSUM with -0.5*Cnorm, then
    16 fp8 DR matmuls accumulate q(x).q(C).  DVE MAX8/FIND_INDEX8 read the
    biased scores straight from PSUM -> per-half top-8 values+indices.
    Host merges the 2x8 candidates (a superset of the global top-8 since
    any global-top-8 score is top-8 within its own half), rescores them in
    exact f32, and fully rescores the ~0.2% of rows whose fp8 top1-top8
    margin is within noise (TAU8) or whose exact top1-top2 margin is a
    rounding-level tie (EPS_TIE), using the reference's jax-on-CPU numerics.
  - "f32r": single-pass fp22-truncated f32 matmul (1 cyc/row) + host fixup
    of rows with top1-top2 margin < TAU (~12 sigma of fp22 noise).
"""

import os
import sys

import numpy as np
import ml_dtypes

for _p in ("/opt/trn_rl_repo",):
    if os.path.isdir(_p) and _p not in sys.path:
        sys.path.insert(0, _p)

import concourse.bass as bass
import concourse.mybir as mybir
import concourse.tile as tile
from concourse import bacc
from concourse.bass_utils import run_bass_kernel_spmd

FP8 = ml_dtypes.float8_e4m3fn

B, T, D, K = 8, 4096, 1024, 4096
N_CORES = 8
ROWS = (B * T) // N_CORES  # 4096 rows per core
P = 128  # SBUF partitions / PE tile
MT = ROWS // P  # 32 row-tiles per core
NB = 512  # one PSUM bank of f32
QC = D // 256  # 4 DoubleRow contraction chunks (256 dims each)
NH = 2  # PSUM half-tiles per row-tile (4 banks each)
KH = K // NH  # 2048 centroids per half
NBH = KH // NB  # 4 PSUM banks per half

DC = D // P  # 8 contraction chunks (f32r mode)
NC_ = K // NB  # 8 centroid chunks (f32r mode)

MODE = os.environ.get("KMEANS_KERNEL_MODE", "fp8dr")
TAU = 0.08  # f32r: score-margin flag threshold (~12 sigma of fp22 noise)
TAU8 = 6.0  # fp8dr: top1-top8 fp8-margin flag threshold (~3.5 sigma)
EPS_TIE = 1e-2  # fp8dr: exact-rescore top1-top2 tie threshold

# Index packing: packed = round((pool*C1 + SHIFT))*1024 + j, exact in f32
# (u = pool*C1 + SHIFT is in [0, 2^14); u*1024 + j < 2^24).  The rounding is
# done by the fp32 add of 2^23 inside the ACT affine (ulp there is 1.0).
C1 = 16.0
SHIFT = float(2**14)
RBIG = float(2**23)
GP = K // 4  # pooled width (stride-GP groups of 4)

_compiled = {}


def _build_fp8dr():
    nc = bacc.Bacc("TRN2", target_bir_lowering=False, debug=False, num_devices=N_CORES)

    x_d = nc.dram_tensor("x", [MT, P, QC, 2, P], mybir.dt.float8e4, kind="ExternalInput")
    c_d = nc.dram_tensor("c", [P, QC, 2, K], mybir.dt.float8e4, kind="ExternalInput")
    bias_d = nc.dram_tensor("bias", [P, K], mybir.dt.float16, kind="ExternalInput")
    delta_d = nc.dram_tensor("delta", [P, P], mybir.dt.float16, kind="ExternalInput")
    iota_d = nc.dram_tensor("iota", [P, GP], mybir.dt.float32, kind="ExternalInput")
    mx_d = nc.dram_tensor("mx", [MT, P, 8], mybir.dt.float32, kind="ExternalOutput")

    with tile.TileContext(nc) as tc:
        with (
            tc.tile_pool(name="const", bufs=1) as cpool,
            tc.tile_pool(name="xp", bufs=3) as xpool,
            tc.tile_pool(name="sc", bufs=2) as spool,
            tc.tile_pool(name="fold", bufs=2) as fpool,
            tc.tile_pool(name="ixp", bufs=6) as ipool,
            tc.tile_pool(name="ps", bufs=2, space=bass.MemorySpace.PSUM) as ppool,
        ):
            # HAM warmup fodder: zeroed fp8 tile, harmless matmuls during DMA wait.
            warm_sb = cpool.tile([P, NB], mybir.dt.float8e4, tag="warm")
            nc.vector.memset(warm_sb[:], 0)

            c_sb = cpool.tile([P, QC, 2, K], mybir.dt.float8e4, tag="c")
            bias_sb = cpool.tile([P, K], mybir.dt.float16, tag="bias")
            delta_sb = cpool.tile([P, P], mybir.dt.float16, tag="delta")
            iota_sb = cpool.tile([P, GP], mybir.dt.float32, tag="iota")
            nc.sync.dma_start(out=delta_sb[:], in_=delta_d[:])
            nc.sync.dma_start(out=bias_sb[:], in_=bias_d[:])
            nc.sync.dma_start(out=iota_sb[:], in_=iota_d[:])
            for c in range(QC):
                nc.sync.dma_start(out=c_sb[:, c], in_=c_d[:, c])

            warm_ps = ppool.tile([P, KH], mybir.dt.float32, tag="ps", name="warm")
            for w in range(24):
                nc.tensor.matmul(
                    warm_ps[:, :NB],
                    warm_sb[:, :P],
                    warm_sb[:],
                    start=True,
                    stop=True,
                )

            for m in range(MT):
                x_sb = xpool.tile([P, QC, 2, P], mybir.dt.float8e4, tag="x")
                nc.sync.dma_start(out=x_sb[:], in_=x_d[m])

                score = spool.tile([P, K], mybir.dt.float32, tag="score")
                for h in range(NH):
                    ps = ppool.tile(
                        [P, KH], mybir.dt.float32, tag="ps", name=f"ps{m}_{h}"
                    )
                    for nb in range(NBH):
                        sl = slice(h * KH + nb * NB, h * KH + (nb + 1) * NB)
                        nc.tensor.matmul(
                            ps[:, nb * NB : (nb + 1) * NB],
                            delta_sb[:],
                            bias_sb[:, sl],
                            start=True,
                            stop=False,
                            skip_group_check=True,
                        )
                    for c in range(QC):
                        for nb in range(NBH):
                            sl = slice(h * KH + nb * NB, h * KH + (nb + 1) * NB)
                            nc.tensor.matmul(
                                ps[:, nb * NB : (nb + 1) * NB],
                                x_sb[:, c],
                                c_sb[:, c, :, sl],
                                start=False,
                                stop=(c == QC - 1),
                                perf_mode=mybir.MatmulPerfMode.DoubleRow,
                                skip_group_check=True,
                            )
                    # ACT drains the biased scores; frees the PSUM banks early
                    nc.scalar.copy(score[:, h * KH : (h + 1) * KH], ps[:])

                # DVE max-folds 4096 -> 1024 (groups of 4, stride GP); the
                # argmax survives folding.  The pooled column index is packed
                # into the value's low bits (ACT affine rounds via +2^23;
                # GPSIMD adds the iota), so a single MAX8 yields both the
                # top-8 pooled scores and their columns -- no FIND_INDEX8.
                t1 = fpool.tile([P, KH], mybir.dt.float32, tag="t1")
                pool = fpool.tile([P, GP], mybir.dt.float32, tag="pool")
                nc.vector.tensor_max(t1[:], score[:, :KH], score[:, KH:])
                nc.vector.tensor_max(pool[:], t1[:, :GP], t1[:, GP:])

                rnd = fpool.tile([P, GP], mybir.dt.float32, tag="rnd")
                pck = fpool.tile([P, GP], mybir.dt.float32, tag="pck")
                nc.scalar.activation(
                    rnd[:], pool[:], mybir.ActivationFunctionType.Copy,
                    scale=C1, bias=RBIG + SHIFT,
                )
                nc.scalar.activation(
                    pck[:], rnd[:], mybir.ActivationFunctionType.Copy,
                    scale=1024.0, bias=-RBIG * 1024.0,
                )
                nc.gpsimd.tensor_add(pck[:], pck[:], iota_sb[:])

                mx = ipool.tile([P, 8], mybir.dt.float32, tag="mx")
                nc.vector.max(out=mx[:], in_=pck[:])
                nc.sync.dma_start(out=mx_d[m], in_=mx[:])

    nc.compile()
    return nc


def _build_f32r():
    nc = bacc.Bacc("TRN2", target_bir_lowering=False, debug=False, num_devices=N_CORES)

    x_d = nc.dram_tensor("x", [MT, DC, P, P], mybir.dt.float32r, kind="ExternalInput")
    c_d = nc.dram_tensor("c", [DC, P, K], mybir.dt.float32r, kind="ExternalInput")
    cn_d = nc.dram_tensor("cn", [P, K], mybir.dt.float32, kind="ExternalInput")
    out_d = nc.dram_tensor("out", [ROWS], mybir.dt.uint32, kind="ExternalOutput")
    marg_d = nc.dram_tensor("marg", [ROWS], mybir.dt.float32, kind="ExternalOutput")

    with tile.TileContext(nc) as tc:
        with (
            tc.tile_pool(name="const", bufs=1) as cpool,
            tc.tile_pool(name="xp", bufs=3) as xpool,
            tc.tile_pool(name="sc", bufs=2) as spool,
            tc.tile_pool(name="ixp", bufs=4) as ipool,
            tc.tile_pool(name="ps", bufs=NC_, space=bass.MemorySpace.PSUM) as ppool,
        ):
            c_sb = cpool.tile([P, DC, K], mybir.dt.float32r, tag="c")
            cn_sb = cpool.tile([P, K], mybir.dt.float32, tag="cn")
            for c in range(DC):
                nc.sync.dma_start(out=c_sb[:, c, :], in_=c_d[c])
            nc.sync.dma_start(out=cn_sb[:], in_=cn_d[:])

            for m in range(MT):
                x_sb = xpool.tile([P, DC, P], mybir.dt.float32r, tag="x")
                nc.sync.dma_start(out=x_sb[:], in_=x_d[m].rearrange("c p j -> p c j"))

                psum_tiles = [
                    ppool.tile([P, NB], mybir.dt.float32, tag="ps", name=f"ps{m}_{n}")
                    for n in range(NC_)
                ]
                for c in range(DC):
                    for n in range(NC_):
                        nc.tensor.matmul(
                            psum_tiles[n][:],
                            x_sb[:, c, :],
                            c_sb[:, c, n * NB : (n + 1) * NB],
                            start=(c == 0),
                            stop=(c == DC - 1),
                        )

                score_sb = spool.tile([P, K], mybir.dt.float32, tag="score")
                for n in range(NC_):
                    sl = slice(n * NB, (n + 1) * NB)
                    nc.scalar.copy(score_sb[:, sl], psum_tiles[n][:])
                    nc.gpsimd.tensor_sub(score_sb[:, sl], score_sb[:, sl], cn_sb[:, sl])

                mx = ipool.tile([P, 8], mybir.dt.float32, tag="mx")
                ix = ipool.tile([P, 8], mybir.dt.uint32, tag="ix")
                mg = ipool.tile([P, 1], mybir.dt.float32, tag="mg")
                nc.vector.max(out=mx[:], in_=score_sb[:])
                nc.vector.max_index(ix[:], mx[:], score_sb[:])
                nc.vector.tensor_sub(mg[:], mx[:, 0:1], mx[:, 1:2])

                nc.sync.dma_start(out=out_d[m * P : (m + 1) * P], in_=ix[:, 0:1])
                nc.sync.dma_start(out=marg_d[m * P : (m + 1) * P], in_=mg[:])

    nc.compile()
    return nc


def _prep_fp8dr(x2, Cf, Cnorm):
    # C: [d, k] -> [p, c, i, k] with d = c*256 + i*128 + p
    cq = np.ascontiguousarray(
        Cf.astype(FP8).reshape(QC, 2, P, K).transpose(2, 0, 1, 3)
    )
    bias = np.zeros((P, K), np.float16)
    bias[0, :] = (-0.5 * Cnorm.reshape(K)).astype(np.float16)
    delta = np.zeros((P, P), np.float16)
    delta[0, :] = 1.0
    iota = np.ascontiguousarray(
        np.broadcast_to(np.arange(GP, dtype=np.float32), (P, GP))
    )

    in_maps = []
    for s in range(N_CORES):
        xs = x2[s * ROWS : (s + 1) * ROWS].astype(FP8)
        # [rows, d] -> [m, p, c, i, j] with rows = m*128 + j, d = c*256 + i*128 + p
        xt = np.ascontiguousarray(
            xs.reshape(MT, P, QC, 2, P).transpose(0, 4, 2, 3, 1)
        )
        in_maps.append({"x": xt, "c": cq, "bias": bias, "delta": delta, "iota": iota})
    return in_maps


def _xt_tiles(xs, dtype):
    # [r, d] -> [m, c, p, j] with r = m*128 + j, d = c*128 + p
    return np.ascontiguousarray(
        xs.astype(dtype).reshape(MT, P, DC, P).transpose(0, 2, 3, 1)
    )


def _prep_f32r(x2, Cf, cn):
    c3 = np.ascontiguousarray(Cf.reshape(DC, P, K))
    in_maps = []
    for s in range(N_CORES):
        xs = x2[s * ROWS : (s + 1) * ROWS]
        in_maps.append({"x": _xt_tiles(xs, np.float32), "c": c3, "cn": cn})
    return in_maps


def _full_rescore(rows, x2, Cf, Cnorm):
    """Exact per-row argmin over all K, replicating the reference's
    jax-on-CPU f32 numerics."""
    import jax
    import jax.numpy as jnp

    cpu = jax.devices("cpu")[0]
    with jax.default_device(cpu):
        xb = jnp.asarray(x2[rows])
        Cj = jnp.asarray(Cf)
        cnj = jnp.asarray(Cnorm.reshape(1, K))
        dist = jnp.sum(xb * xb, axis=1, keepdims=True) - 2.0 * (xb @ Cj) + cnj
        return np.asarray(jnp.argmin(dist, axis=1))


def _decode_fp8dr(res, x2, Cf, Cnorm):
    N = B * T
    packed = np.concatenate(
        [np.asarray(res.results[s]["mx"]).reshape(ROWS, 8) for s in range(N_CORES)]
    )  # [N, 8] packed (quantized pooled value, pooled column), descending
    pi = np.round(packed.astype(np.float64)).astype(np.int64)
    g8 = pi % GP  # pooled columns of the top-8 groups
    np.clip(g8, 0, GP - 1, out=g8)
    vals = ((pi // GP).astype(np.float64) - SHIFT) / C1  # quantized pooled scores
    # candidates: the full stride-groups of the top-8 pooled columns (the
    # group winner plus its mates, any of which can be the exact argmin).
    cands = (g8[:, :, None] + (np.arange(K // GP) * GP)[None, None, :]).reshape(N, -1)

    # exact f32 rescore of the candidates, in distance space
    CT = np.ascontiguousarray(Cf.T)  # [K, D]
    cn = Cnorm.reshape(K)
    pick = np.zeros(N, np.int64)
    tie = np.zeros(N, np.float32)
    CH = 4096
    for i in range(0, N, CH):
        cd = cands[i : i + CH]
        d = cn[cd] - 2.0 * np.einsum(
            "nkd,nd->nk", CT[cd], x2[i : i + CH], optimize=True
        )
        j = np.argmin(d, axis=1)
        pick[i : i + CH] = cd[np.arange(len(cd)), j]
        ds = np.sort(d, axis=1)
        tie[i : i + CH] = ds[:, 1] - ds[:, 0]

    # flag rows where fp8 noise or f32 rounding could flip the argmin
    gap8 = vals[:, 0] - vals[:, 7]
    flagged = np.flatnonzero((gap8 < TAU8) | (tie < EPS_TIE))
    if flagged.size:
        pick[flagged] = _full_rescore(flagged, x2, Cf, Cnorm)
    return pick


def _host_fixup_f32r(assigned, margins, x2, Cf, Cnorm):
    bad = np.flatnonzero(margins < TAU)
    if bad.size == 0:
        return assigned
    assigned[bad] = _full_rescore(bad, x2, Cf, Cnorm).astype(assigned.dtype)
    return assigned


def run(inputs, trace=False, mode=None):
    """Returns (assigned [B, T] int32, BassKernelResults)."""
    mode = mode or MODE
    if mode not in _compiled:
        _compiled[mode] = _build_fp8dr() if mode == "fp8dr" else _build_f32r()
    nc = _compiled[mode]

    x2 = np.ascontiguousarray(
        np.asarray(inputs["x"], dtype=np.float32).reshape(B * T, D)
    )
    Cf = np.ascontiguousarray(np.asarray(inputs["C"], dtype=np.float32))
    Cnorm = np.asarray(inputs["Cnorm"], dtype=np.float32)

    if mode == "fp8dr":
        in_maps = _prep_fp8dr(x2, Cf, Cnorm)
    else:
        cn = np.ascontiguousarray(
            np.broadcast_to(0.5 * Cnorm.reshape(1, K), (P, K)).astype(np.float32)
        )
        in_maps = _prep_f32r(x2, Cf, cn)

    res = run_bass_kernel_spmd(nc, in_maps, list(range(N_CORES)), trace=trace)

    if mode == "fp8dr":
        assigned = _decode_fp8dr(res, x2, Cf, Cnorm).astype(np.int32)
    else:
        assigned = np.concatenate(
            [np.asarray(res.results[s]["out"]).reshape(ROWS) for s in range(N_CORES)]
        ).astype(np.int32)
        margins = np.concatenate(
            [np.asarray(res.results[s]["marg"]).reshape(ROWS) for s in range(N_CORES)]
        )
        assigned = _host_fixup_f32r(assigned, margins, x2, Cf, Cnorm)
    return assigned.reshape(B, T), res


def kernel(x, C, Cnorm):
    assigned, _ = run({"x": x, "C": C, "Cnorm": Cnorm})
    return assigned
